# revision 60
# baseline (speedup 1.0000x reference)
"""Local (windowed) attention with shared KV head — TRN2 Bass kernel.

Problem: b=1, L=4096, d_model=1024, n_head=16, d_head=64, w=512.
  qp = (q@Wq)/8; k,v = kv@Wkv; per 512-chunk attention over {prev,self,next}
  chunks with zero-padded edges (softmax includes exp(0)=1 terms for pads);
  out = ctx @ Wo.

Sharding: sequence-parallel over the 8 chunks, one chunk per NeuronCore.
Each core recomputes the K/V projection for its 3-chunk halo (no
collectives). Edge cores receive zero-filled halo slices, which reproduces
the reference's zero-padding exactly (scores 0 -> exp 1 in the softmax).

All matmuls bf16 (1 cycle/row at 2.4 GHz; fp32 PSUM accumulation). The
steady state is ScalarE-paced: one exp ACTIVATE [128,1024] per y-tile
(~1.15us) against ~0.65us of PE work, so the kernel is structured to keep
the ACTIVATE stream gapless:
  - dummy warmup matmuls open the HAM clock gate during the DMA fill
  - input DMAs spread over 3 queues (sync / scalar / gpsimd)
  - kv-proj and q-proj interleave; q-proj tiles m>=2 are computed inside
    the attention loop (2 pairs ahead) so attention starts early
  - scores for a head pair interleave into one psum tile; the two matmuls
    use disjoint PE row groups and run concurrently
  - softmax denominators batch through one vector.reciprocal per batch
    (heads 0-11 while pairs 6-7 run; heads 12-15 overlap the out-proj)
  - out-proj runs i-outer in two 4-bank halves right behind the ctxn muls
"""

import numpy as np

B, L, DM, NH, DH, W = 1, 4096, 1024, 16, 64, 512
NCORES = 8
CH = L // NCORES        # 512 tokens per core
YW = 3 * W              # 1536 halo positions
P = 128
NF = DM // P            # 8 feature tiles
NY = YW // P            # 12 y tiles
NPAIR = NH // 2         # 8 head pairs

_CACHE = {}


def _zrow(hh):
    # heads 12-15 live at partitions 32-35 so the late reciprocal batch
    # starts at a 32-aligned partition base
    return hh if hh < 12 else 32 + (hh - 12)


def _build():
    import concourse.mybir as mybir
    import concourse.tile as tile
    from concourse import bacc
    from concourse.masks import make_identity
    from contextlib import ExitStack

    F32 = mybir.dt.float32
    BF16 = mybir.dt.bfloat16
    EXP = mybir.ActivationFunctionType.Exp

    nc = bacc.Bacc("TRN2", target_bir_lowering=False, debug=False)
    QT = nc.dram_tensor("QT", [DM, CH], BF16, kind="ExternalInput")
    ESEL = nc.dram_tensor("ESEL", [36, NH * 64], BF16, kind="ExternalInput")
    KVT = nc.dram_tensor("KVT", [DM, YW], BF16, kind="ExternalInput")
    WQ = nc.dram_tensor("WQ", [DM, DM], BF16, kind="ExternalInput")    # pre-scaled by 1/8
    WVK = nc.dram_tensor("WVK", [DM, P], BF16, kind="ExternalInput")   # [Wv | Wk]
    WO = nc.dram_tensor("WO", [DM, DM], BF16, kind="ExternalInput")
    OUT = nc.dram_tensor("OUT", [CH, DM], F32, kind="ExternalOutput")

    with tile.TileContext(nc) as tc, ExitStack() as ctx:
        perm = ctx.enter_context(tc.tile_pool(name="perm", bufs=1))

        identb = perm.tile([64, 64], F32, tag="identb")
        make_identity(nc, identb[:])
        esel = perm.tile([36, NH * 64], BF16, tag="esel")

        # --- persistent SBUF tiles
        wvk = [perm.tile([P, P], BF16, tag=f"wvk{f}", name=f"wvk{f}") for f in range(NF)]
        wq = [perm.tile([P, DM], BF16, tag=f"wq{f}", name=f"wq{f}") for f in range(NF)]
        wo = [perm.tile([P, DM], BF16, tag=f"wo{f}", name=f"wo{f}") for f in range(NF)]
        k3T2 = perm.tile([P, YW], BF16, tag="k3T2")
        vTs = perm.tile([64, YW], F32, tag="vTs")
        v65 = [perm.tile([P, 65], BF16, tag=f"v65_{t}", name=f"v65_{t}") for t in range(NY)]
        qpT = [perm.tile([P, CH], BF16, tag=f"qpT{m}", name=f"qpT{m}") for m in range(NF)]
        ctxn = [perm.tile([P, CH], BF16, tag=f"ctxn{i}", name=f"ctxn{i}") for i in range(NPAIR)]
        cxs = [perm.tile([64, W], BF16, tag=f"cxs{h}", name=f"cxs{h}") for h in range(NH)]
        zr16 = perm.tile([36, W], F32, tag="zr16")
        zi16 = perm.tile([36, W], F32, tag="zi16")
        zi16b = perm.tile([36, W], BF16, tag="zi16b")

        # HAM warmup: dense dummy matmuls during the DMA fill open the PE
        # clock gate (needs ~3.4us of sustained activity)
        wtile = perm.tile([P, W], BF16, tag="wtile")
        nc.vector.memset(wtile[:], 1.0)
        with tc.tile_pool(name="wmps", bufs=1, space="PSUM") as wmp:
            wps = wmp.tile([P, W], F32, tag="wm")
            for _ in range(9):
                nc.tensor.matmul(wps[:], wtile[:, 0:P], wtile[:],
                                 start=True, stop=True)

        with tc.tile_pool(name="qt", bufs=1) as qtp, \
             tc.tile_pool(name="qpps", bufs=1, space="PSUM") as qpp, \
             tc.tile_pool(name="zn", bufs=6) as znp:
            qt = [qtp.tile([P, CH], BF16, tag=f"qt{f}", name=f"qt{f}") for f in range(NF)]

            # --- input DMA schedule, 3 queues
            # gpsimd: wvk -> qt -> wq (per output-column slice) -> esel
            for f in range(NF):
                nc.gpsimd.dma_start(wvk[f][:], WVK.ap()[P * f:P * (f + 1), :])
            for f in range(NF):
                nc.gpsimd.dma_start(qt[f][:], QT.ap()[P * f:P * (f + 1), :])
            for m in range(NF):
                ms = slice(P * m, P * (m + 1))
                for f in range(NF):
                    nc.gpsimd.dma_start(wq[f][:, ms], WQ.ap()[P * f:P * (f + 1), ms])
            nc.gpsimd.dma_start(esel[:], ESEL.ap()[:, :])

            def qproj(m):
                ps = qpp.tile([P, CH], F32, tag="qp")
                for f in range(NF):
                    nc.tensor.matmul(ps[:], wq[f][:, P * m:P * (m + 1)], qt[f][:],
                                     start=(f == 0), stop=(f == NF - 1))
                with nc.allow_low_precision(reason="bf16 attention pipeline"):
                    nc.vector.tensor_copy(qpT[m][:], ps[:])

            with tc.tile_pool(name="kvt", bufs=1) as kvtp, \
                 tc.tile_pool(name="ph0ps", bufs=3, space="PSUM") as ph0, \
                 tc.tile_pool(name="tpps", bufs=2, space="PSUM") as tpp:
                kvt = [kvtp.tile([P, YW], BF16, tag=f"kvt{f}", name=f"kvt{f}")
                       for f in range(NF)]
                # kv halo loads: low feature-half on sync, high half on the
                # scalar hwdge queue, ordered by w-chunk
                for n in range(3):
                    ns_ = slice(W * n, W * (n + 1))
                    for f in range(NF // 2):
                        nc.sync.dma_start(kvt[f][:, ns_],
                                          KVT.ap()[P * f:P * (f + 1), ns_])
                    for f in range(NF // 2, NF):
                        nc.scalar.dma_start(kvt[f][:, ns_],
                                            KVT.ap()[P * f:P * (f + 1), ns_])

                def kvproj(n):
                    ps = ph0.tile([P, W], F32, tag="kvp")
                    for f in range(NF):
                        nc.tensor.matmul(ps[:], wvk[f][:],
                                         kvt[f][:, W * n:W * (n + 1)],
                                         start=(f == 0), stop=(f == NF - 1))
                    ns = slice(W * n, W * (n + 1))
                    with nc.allow_low_precision(reason="bf16 attention pipeline"):
                        nc.vector.tensor_copy(vTs[:, ns], ps[0:64, :])
                        nc.vector.tensor_copy(k3T2[64:128, ns], ps[64:128, :])

                # interleave kv-proj w-chunks with the first q-proj tiles
                kvproj(0)
                qproj(0)
                kvproj(1)
                qproj(1)
                kvproj(2)
                # duplicate kT into the low partition half (partition remap)
                nc.sync.dma_start(k3T2[0:64, :], k3T2[64:128, :])
                # v65 tiles: PE transpose of vT + ones column
                for t in range(NY):
                    tp = tpp.tile([P, 64], F32, tag="tp")
                    nc.tensor.transpose(tp[:], vTs[:, P * t:P * (t + 1)],
                                        identb[:])
                    with nc.allow_low_precision(reason="bf16 attention pipeline"):
                        nc.vector.tensor_copy(v65[t][:, 0:64], tp[:])
                    nc.vector.memset(v65[t][:, 64:65], 1.0)

            # out-proj weights arrive during attention on the scalar queue
            for f in range(NF):
                nc.scalar.dma_start(wo[f][:], WO.ap()[P * f:P * (f + 1), :])

            def z_recip(heads):
                lo, hi = _zrow(heads[0]), _zrow(heads[-1]) + 1
                with nc.allow_low_precision(reason="softmax denom"):
                    nc.vector.reciprocal(zi16[lo:hi, :], zr16[lo:hi, :])
                    nc.vector.tensor_copy(zi16b[lo:hi, :], zi16[lo:hi, :])

            def z_apply(heads):
                lo, hi = _zrow(heads[0]), _zrow(heads[-1]) + 1
                for hh in heads:
                    i, h = hh // 2, hh % 2
                    zb = qpp.tile([P, W], F32, tag="qp")
                    nc.tensor.matmul(zb[0:64, :],
                                     esel[lo:hi, 64 * hh:64 * (hh + 1)],
                                     zi16b[lo:hi, :], start=True, stop=True)
                    if h == 0:
                        with nc.allow_low_precision(reason="bf16 ctx"):
                            nc.vector.tensor_mul(ctxn[i][0:64, :], cxs[hh][:],
                                                 zb[0:64, :])
                    else:
                        cbt = znp.tile([64, W], BF16, tag="cbt")
                        with nc.allow_low_precision(reason="bf16 ctx"):
                            nc.vector.tensor_mul(cbt[:], cxs[hh][:], zb[0:64, :])
                        nc.sync.dma_start(ctxn[i][64:128, :], cbt[:])

            # --- attention per head pair; scores for the two heads interleave
            # into one psum tile (A cols 0:512 rows-grp 0:64, B cols 512:1024
            # row-grp 64:128) so the row-tiled matmuls run concurrently
            attn = ExitStack()
            scp = attn.enter_context(tc.tile_pool(name="scps", bufs=2, space="PSUM"))
            cxp = attn.enter_context(tc.tile_pool(name="cxps", bufs=3, space="PSUM"))
            ptp = attn.enter_context(tc.tile_pool(name="pt", bufs=4))
            for i in range(NPAIR):
                cxA = cxp.tile([P, W], F32, tag="cx")
                cxB = cxp.tile([P, W], F32, tag="cx")
                for y in range(NY):
                    ys = slice(P * y, P * (y + 1))
                    sc = scp.tile([P, 2 * W], F32, tag="sc")
                    nc.tensor.matmul(sc[:, 0:W], k3T2[0:64, ys],
                                     qpT[i][0:64, :], start=True, stop=True,
                                     tile_position=(0, 0))
                    nc.tensor.matmul(sc[:, W:2 * W], k3T2[64:128, ys],
                                     qpT[i][64:128, :], start=True, stop=True,
                                     tile_position=(64, 0))
                    pab = ptp.tile([P, 2 * W], BF16, tag="pt")
                    with nc.allow_low_precision(reason="bf16 probs"):
                        nc.scalar.activation(pab[:], sc[:], EXP)
                    st = (y == 0)
                    sp = (y == NY - 1)
                    nc.tensor.matmul(cxA[0:65, :], v65[y][:], pab[:, 0:W],
                                     start=st, stop=sp)
                    nc.tensor.matmul(cxB[0:65, :], v65[y][:], pab[:, W:2 * W],
                                     start=st, stop=sp)
                # stage Z row + unnormalized ctx out of PSUM (frees cx banks)
                for h, cx in ((0, cxA), (1, cxB)):
                    hh = 2 * i + h
                    zt = znp.tile([65, W], F32, tag="zt")
                    nc.vector.tensor_copy(zt[64:65, :], cx[64:65, :])
                    nc.sync.dma_start(zr16[_zrow(hh):_zrow(hh) + 1, :],
                                      zt[64:65, :])
                    with nc.allow_low_precision(reason="bf16 ctx"):
                        nc.vector.tensor_copy(cxs[hh][:], cx[0:64, :])
                if i + 2 < NF:
                    qproj(i + 2)
                if i == 5:
                    z_recip(list(range(12)))     # overlaps pair 6
                if i == 6:
                    z_apply(list(range(12)))     # overlaps pair 7
            attn.close()
            # heads 12-15: reciprocal overlaps the first out-proj matmuls
            z_recip([12, 13, 14, 15])

            # --- output projection, in two 4-bank halves; i-outer so the
            # first 7 pairs' matmuls don't wait on pair 7's ctxn
            with tc.tile_pool(name="opps", bufs=4, space="PSUM") as opp, \
                 tc.tile_pool(name="osb", bufs=4) as osb:
                allblk = [(x, o) for x in range(4) for o in range(2)]

                def oproj(pso, blocks, irange):
                    for i in irange:
                        for ps, (x, o) in zip(pso, blocks):
                            xs = slice(P * x, P * (x + 1))
                            os_ = slice(W * o, W * (o + 1))
                            nc.tensor.matmul(ps[:], ctxn[i][:, xs],
                                             wo[i][:, os_],
                                             start=(i == 0),
                                             stop=(i == NPAIR - 1))

                blocks0 = allblk[0:4]
                pso0 = [opp.tile([P, W], F32, tag="op", name=f"op0_{b}")
                        for b in range(4)]
                oproj(pso0, blocks0, range(6))
                z_apply([12, 13, 14, 15])
                oproj(pso0, blocks0, range(6, NPAIR))
                for ps, (x, o) in zip(pso0, blocks0):
                    ot = osb.tile([P, W], F32, tag="os", name=f"ot0_{x}_{o}")
                    nc.scalar.copy(ot[:], ps[:])
                    nc.sync.dma_start(OUT.ap()[P * x:P * (x + 1),
                                               W * o:W * (o + 1)], ot[:])
                blocks1 = allblk[4:8]
                pso1 = [opp.tile([P, W], F32, tag="op", name=f"op1_{b}")
                        for b in range(4)]
                oproj(pso1, blocks1, range(NPAIR))
                for ps, (x, o) in zip(pso1, blocks1):
                    ot = osb.tile([P, W], F32, tag="os", name=f"ot1_{x}_{o}")
                    nc.scalar.copy(ot[:], ps[:])
                    nc.sync.dma_start(OUT.ap()[P * x:P * (x + 1),
                                               W * o:W * (o + 1)], ot[:])

    nc.compile()
    return nc


def _get_nc():
    if "nc" not in _CACHE:
        _CACHE["nc"] = _build()
    return _CACHE["nc"]


def _esel():
    import ml_dtypes
    e = np.zeros((36, NH * 64), ml_dtypes.bfloat16)
    for h in range(NH):
        e[_zrow(h), 64 * h:64 * (h + 1)] = 1.0
    return e


def kernel(q, kv, Wq, Wkv, Wo, w=None, _trace=False):
    from concourse import bass_utils
    import ml_dtypes

    BF = ml_dtypes.bfloat16

    q = np.asarray(q, np.float32).reshape(L, DM)
    kv = np.asarray(kv, np.float32).reshape(L, DM)
    Wq = np.asarray(Wq, np.float32)
    Wkv = np.asarray(Wkv, np.float32)
    Wo = np.asarray(Wo, np.float32)

    qT = np.ascontiguousarray(q.T).astype(BF)           # [DM, L]
    kvT = np.ascontiguousarray(kv.T).astype(BF)         # [DM, L]
    WQs = np.ascontiguousarray(Wq / np.sqrt(DH)).astype(BF)   # fold 1/sqrt(d_head)
    WVK = np.ascontiguousarray(
        np.concatenate([Wkv[:, DH:], Wkv[:, :DH]], axis=1)).astype(BF)  # [Wv | Wk]
    WOb = np.ascontiguousarray(Wo).astype(BF)

    in_maps = []
    for c in range(NCORES):
        kvt_c = np.zeros((DM, YW), BF)
        lo = (c - 1) * CH
        hi = (c + 2) * CH
        src_lo, src_hi = max(lo, 0), min(hi, L)
        dst_lo = src_lo - lo
        kvt_c[:, dst_lo:dst_lo + (src_hi - src_lo)] = kvT[:, src_lo:src_hi]
        in_maps.append({
            "QT": np.ascontiguousarray(qT[:, c * CH:(c + 1) * CH]),
            "KVT": kvt_c,
            "WQ": WQs,
            "WVK": WVK,
            "WO": WOb,
            "ESEL": _esel(),
        })

    nc = _get_nc()
    res = bass_utils.run_bass_kernel_spmd(
        nc, in_maps, core_ids=list(range(NCORES)), trace=_trace)
    if _trace:
        _CACHE["last_result"] = res

    out = np.concatenate([r["OUT"] for r in res.results], axis=0)
    return out.reshape(B, L, DM).astype(np.float32)


# revision 65
# speedup vs baseline: 1.0019x; 1.0019x over previous
"""Local (windowed) attention with shared KV head — TRN2 Bass kernel.

Problem: b=1, L=4096, d_model=1024, n_head=16, d_head=64, w=512.
  qp = (q@Wq)/8; k,v = kv@Wkv; per 512-chunk attention over {prev,self,next}
  chunks with zero-padded edges (softmax includes exp(0)=1 terms for pads);
  out = ctx @ Wo.

Sharding: sequence-parallel over the 8 chunks, one chunk per NeuronCore.
Each core recomputes the K/V projection for its 3-chunk halo (no
collectives). Edge cores receive zero-filled halo slices, which reproduces
the reference's zero-padding exactly (scores 0 -> exp 1 in the softmax).

All matmuls bf16 (1 cycle/row at 2.4 GHz; fp32 PSUM accumulation). The
steady state is ScalarE-paced: one exp ACTIVATE [128,1024] per y-tile
(~1.15us) against ~0.65us of PE work, so the kernel is structured to keep
the ACTIVATE stream gapless:
  - dummy warmup matmuls open the HAM clock gate during the DMA fill
  - input DMAs spread over 3 queues (sync / scalar / gpsimd)
  - kv-proj and q-proj interleave; q-proj tiles m>=2 are computed inside
    the attention loop (2 pairs ahead) so attention starts early
  - scores for a head pair interleave into one psum tile; the two matmuls
    use disjoint PE row groups and run concurrently
  - softmax denominators batch through one vector.reciprocal per batch
    (heads 0-11 while pairs 6-7 run; heads 12-15 overlap the out-proj)
  - out-proj runs i-outer in two 4-bank halves right behind the ctxn muls
"""

import numpy as np

B, L, DM, NH, DH, W = 1, 4096, 1024, 16, 64, 512
NCORES = 8
CH = L // NCORES        # 512 tokens per core
YW = 3 * W              # 1536 halo positions
P = 128
NF = DM // P            # 8 feature tiles
NY = YW // P            # 12 y tiles
NPAIR = NH // 2         # 8 head pairs

_CACHE = {}


def _zrow(hh):
    # heads 12-15 live at partitions 32-35 so the late reciprocal batch
    # starts at a 32-aligned partition base
    return hh if hh < 12 else 32 + (hh - 12)


def _build():
    import concourse.mybir as mybir
    import concourse.tile as tile
    from concourse import bacc
    from concourse.masks import make_identity
    from contextlib import ExitStack

    F32 = mybir.dt.float32
    BF16 = mybir.dt.bfloat16
    I16 = mybir.dt.int16
    EXP = mybir.ActivationFunctionType.Exp
    # Schraudolph: e^x ~= bf16_bits(round(x * 128/ln2 + (127*128 - 7)))
    SCHRA_A = 128.0 / float(np.log(2.0))
    SCHRA_B = 127.0 * 128.0 - 7.0

    nc = bacc.Bacc("TRN2", target_bir_lowering=False, debug=False)
    QT = nc.dram_tensor("QT", [DM, CH], BF16, kind="ExternalInput")
    ESEL = nc.dram_tensor("ESEL", [36, NH * 64], BF16, kind="ExternalInput")
    KVT = nc.dram_tensor("KVT", [DM, YW], BF16, kind="ExternalInput")
    WQ = nc.dram_tensor("WQ", [DM, DM], BF16, kind="ExternalInput")    # pre-scaled by 1/8
    WVK = nc.dram_tensor("WVK", [DM, P], BF16, kind="ExternalInput")   # [Wv | Wk]
    WO = nc.dram_tensor("WO", [DM, DM], BF16, kind="ExternalInput")
    OUT = nc.dram_tensor("OUT", [CH, DM], F32, kind="ExternalOutput")

    with tile.TileContext(nc) as tc, ExitStack() as ctx:
        perm = ctx.enter_context(tc.tile_pool(name="perm", bufs=1))

        identb = perm.tile([64, 64], F32, tag="identb")
        make_identity(nc, identb[:])
        esel = perm.tile([36, NH * 64], BF16, tag="esel")

        # --- persistent SBUF tiles
        wvk = [perm.tile([P, P], BF16, tag=f"wvk{f}", name=f"wvk{f}") for f in range(NF)]
        wq = [perm.tile([P, DM], BF16, tag=f"wq{f}", name=f"wq{f}") for f in range(NF)]
        wo = [perm.tile([P, DM], BF16, tag=f"wo{f}", name=f"wo{f}") for f in range(NF)]
        k3T2 = perm.tile([P, YW], BF16, tag="k3T2")
        vTs = perm.tile([64, YW], F32, tag="vTs")
        v65 = [perm.tile([P, 65], BF16, tag=f"v65_{t}", name=f"v65_{t}") for t in range(NY)]
        qpT = [perm.tile([P, CH], BF16, tag=f"qpT{m}", name=f"qpT{m}") for m in range(NF)]
        ctxn = [perm.tile([P, CH], BF16, tag=f"ctxn{i}", name=f"ctxn{i}") for i in range(NPAIR)]
        cxs = [perm.tile([64, W], BF16, tag=f"cxs{h}", name=f"cxs{h}") for h in range(NH)]
        zr16 = perm.tile([36, W], F32, tag="zr16")
        zi16 = perm.tile([36, W], F32, tag="zi16")
        zi16b = perm.tile([36, W], BF16, tag="zi16b")

        # HAM warmup: dense dummy matmuls during the DMA fill open the PE
        # clock gate (needs ~3.4us of sustained activity)
        wtile = perm.tile([P, W], BF16, tag="wtile")
        nc.vector.memset(wtile[:], 1.0)
        with tc.tile_pool(name="wmps", bufs=1, space="PSUM") as wmp:
            wps = wmp.tile([P, W], F32, tag="wm")
            for _ in range(9):
                nc.tensor.matmul(wps[:], wtile[:, 0:P], wtile[:],
                                 start=True, stop=True)

        with tc.tile_pool(name="qt", bufs=1) as qtp, \
             tc.tile_pool(name="qpps", bufs=1, space="PSUM") as qpp, \
             tc.tile_pool(name="zn", bufs=6) as znp:
            qt = [qtp.tile([P, CH], BF16, tag=f"qt{f}", name=f"qt{f}") for f in range(NF)]

            # --- input DMA schedule, 3 queues
            # gpsimd: wvk -> qt -> wq (per output-column slice) -> esel
            for f in range(NF):
                nc.gpsimd.dma_start(wvk[f][:], WVK.ap()[P * f:P * (f + 1), :])
            for f in range(NF):
                nc.gpsimd.dma_start(qt[f][:], QT.ap()[P * f:P * (f + 1), :])
            for m in range(NF):
                ms = slice(P * m, P * (m + 1))
                for f in range(NF):
                    nc.gpsimd.dma_start(wq[f][:, ms], WQ.ap()[P * f:P * (f + 1), ms])
            nc.gpsimd.dma_start(esel[:], ESEL.ap()[:, :])

            def qproj(m):
                ps = qpp.tile([P, CH], F32, tag="qp")
                for f in range(NF):
                    nc.tensor.matmul(ps[:], wq[f][:, P * m:P * (m + 1)], qt[f][:],
                                     start=(f == 0), stop=(f == NF - 1))
                with nc.allow_low_precision(reason="bf16 attention pipeline"):
                    nc.vector.tensor_copy(qpT[m][:], ps[:])

            with tc.tile_pool(name="kvt", bufs=1) as kvtp, \
                 tc.tile_pool(name="ph0ps", bufs=3, space="PSUM") as ph0, \
                 tc.tile_pool(name="tpps", bufs=2, space="PSUM") as tpp:
                kvt = [kvtp.tile([P, YW], BF16, tag=f"kvt{f}", name=f"kvt{f}")
                       for f in range(NF)]
                # kv halo loads: low feature-half on sync, high half on the
                # scalar hwdge queue, ordered by w-chunk
                for n in range(3):
                    ns_ = slice(W * n, W * (n + 1))
                    for f in range(NF // 2):
                        nc.sync.dma_start(kvt[f][:, ns_],
                                          KVT.ap()[P * f:P * (f + 1), ns_])
                    for f in range(NF // 2, NF):
                        nc.scalar.dma_start(kvt[f][:, ns_],
                                            KVT.ap()[P * f:P * (f + 1), ns_])

                def kvproj(n):
                    ps = ph0.tile([P, W], F32, tag="kvp")
                    for f in range(NF):
                        nc.tensor.matmul(ps[:], wvk[f][:],
                                         kvt[f][:, W * n:W * (n + 1)],
                                         start=(f == 0), stop=(f == NF - 1))
                    ns = slice(W * n, W * (n + 1))
                    with nc.allow_low_precision(reason="bf16 attention pipeline"):
                        nc.vector.tensor_copy(vTs[:, ns], ps[0:64, :])
                        nc.vector.tensor_copy(k3T2[64:128, ns], ps[64:128, :])

                # interleave kv-proj w-chunks with the first q-proj tiles
                kvproj(0)
                qproj(0)
                kvproj(1)
                qproj(1)
                kvproj(2)
                # duplicate kT into the low partition half (partition remap)
                nc.sync.dma_start(k3T2[0:64, :], k3T2[64:128, :])
                # v65 tiles: PE transpose of vT + ones column
                for t in range(NY):
                    tp = tpp.tile([P, 64], F32, tag="tp")
                    nc.tensor.transpose(tp[:], vTs[:, P * t:P * (t + 1)],
                                        identb[:])
                    with nc.allow_low_precision(reason="bf16 attention pipeline"):
                        nc.vector.tensor_copy(v65[t][:, 0:64], tp[:])
                    nc.vector.memset(v65[t][:, 64:65], 1.0)

            # out-proj weights arrive during attention on the scalar queue
            for f in range(NF):
                nc.scalar.dma_start(wo[f][:], WO.ap()[P * f:P * (f + 1), :])

            def z_recip(heads):
                lo, hi = _zrow(heads[0]), _zrow(heads[-1]) + 1
                with nc.allow_low_precision(reason="softmax denom"):
                    nc.vector.reciprocal(zi16[lo:hi, :], zr16[lo:hi, :])
                    nc.vector.tensor_copy(zi16b[lo:hi, :], zi16[lo:hi, :])

            def z_apply(heads):
                lo, hi = _zrow(heads[0]), _zrow(heads[-1]) + 1
                for hh in heads:
                    i, h = hh // 2, hh % 2
                    zb = qpp.tile([P, W], F32, tag="qp")
                    nc.tensor.matmul(zb[0:64, :],
                                     esel[lo:hi, 64 * hh:64 * (hh + 1)],
                                     zi16b[lo:hi, :], start=True, stop=True)
                    if h == 0:
                        with nc.allow_low_precision(reason="bf16 ctx"):
                            nc.vector.tensor_mul(ctxn[i][0:64, :], cxs[hh][:],
                                                 zb[0:64, :])
                    else:
                        cbt = znp.tile([64, W], BF16, tag="cbt")
                        with nc.allow_low_precision(reason="bf16 ctx"):
                            nc.vector.tensor_mul(cbt[:], cxs[hh][:], zb[0:64, :])
                        nc.sync.dma_start(ctxn[i][64:128, :], cbt[:])

            # --- attention per head pair; scores for the two heads interleave
            # into one psum tile (A cols 0:512 rows-grp 0:64, B cols 512:1024
            # row-grp 64:128) so the row-tiled matmuls run concurrently
            attn = ExitStack()
            scp = attn.enter_context(tc.tile_pool(name="scps", bufs=2, space="PSUM"))
            cxp = attn.enter_context(tc.tile_pool(name="cxps", bufs=3, space="PSUM"))
            ptp = attn.enter_context(tc.tile_pool(name="pt", bufs=4))
            for i in range(NPAIR):
                cxA = cxp.tile([P, W], F32, tag="cx")
                cxB = cxp.tile([P, W], F32, tag="cx")
                for y in range(NY):
                    ys = slice(P * y, P * (y + 1))
                    sc = scp.tile([P, 2 * W], F32, tag="sc")
                    nc.tensor.matmul(sc[:, 0:W], k3T2[0:64, ys],
                                     qpT[i][0:64, :], start=True, stop=True,
                                     tile_position=(0, 0))
                    nc.tensor.matmul(sc[:, W:2 * W], k3T2[64:128, ys],
                                     qpT[i][64:128, :], start=True, stop=True,
                                     tile_position=(64, 0))
                    if y == NY - 1:
                        # last y-tile per pair: DVE fast-exp (one fused
                        # x*a+b with round-to-int16, bitcast to bf16) — its
                        # longer latency hides at the pair boundary and the
                        # ScalarE exp stream (the pacing engine) shortens
                        pabi = ptp.tile([P, 2 * W], I16, tag="pti")
                        with nc.allow_low_precision(reason="fast-exp probs"):
                            nc.vector.tensor_scalar(
                                pabi[:], sc[:], SCHRA_A, SCHRA_B,
                                mybir.AluOpType.mult, mybir.AluOpType.add)
                        pab = pabi[:].bitcast(BF16)
                    else:
                        pabt = ptp.tile([P, 2 * W], BF16, tag="pt")
                        with nc.allow_low_precision(reason="bf16 probs"):
                            nc.scalar.activation(pabt[:], sc[:], EXP)
                        pab = pabt[:]
                    st = (y == 0)
                    sp = (y == NY - 1)
                    nc.tensor.matmul(cxA[0:65, :], v65[y][:], pab[:, 0:W],
                                     start=st, stop=sp)
                    nc.tensor.matmul(cxB[0:65, :], v65[y][:], pab[:, W:2 * W],
                                     start=st, stop=sp)
                # stage Z row + unnormalized ctx out of PSUM (frees cx banks)
                for h, cx in ((0, cxA), (1, cxB)):
                    hh = 2 * i + h
                    zt = znp.tile([65, W], F32, tag="zt")
                    nc.vector.tensor_copy(zt[64:65, :], cx[64:65, :])
                    nc.sync.dma_start(zr16[_zrow(hh):_zrow(hh) + 1, :],
                                      zt[64:65, :])
                    with nc.allow_low_precision(reason="bf16 ctx"):
                        nc.vector.tensor_copy(cxs[hh][:], cx[0:64, :])
                if i + 2 < NF:
                    qproj(i + 2)
                if i == 5:
                    z_recip(list(range(12)))     # overlaps pair 6
                if i == 6:
                    z_apply(list(range(12)))     # overlaps pair 7
            attn.close()
            # heads 12-15: reciprocal overlaps the first out-proj matmuls
            z_recip([12, 13, 14, 15])

            # --- output projection, in two 4-bank halves; i-outer so the
            # first 7 pairs' matmuls don't wait on pair 7's ctxn
            with tc.tile_pool(name="opps", bufs=4, space="PSUM") as opp, \
                 tc.tile_pool(name="osb", bufs=4) as osb:
                allblk = [(x, o) for x in range(4) for o in range(2)]

                def oproj(pso, blocks, irange):
                    for i in irange:
                        for ps, (x, o) in zip(pso, blocks):
                            xs = slice(P * x, P * (x + 1))
                            os_ = slice(W * o, W * (o + 1))
                            nc.tensor.matmul(ps[:], ctxn[i][:, xs],
                                             wo[i][:, os_],
                                             start=(i == 0),
                                             stop=(i == NPAIR - 1))

                blocks0 = allblk[0:4]
                pso0 = [opp.tile([P, W], F32, tag="op", name=f"op0_{b}")
                        for b in range(4)]
                oproj(pso0, blocks0, range(6))
                z_apply([12, 13, 14, 15])
                oproj(pso0, blocks0, range(6, NPAIR))
                for ps, (x, o) in zip(pso0, blocks0):
                    ot = osb.tile([P, W], F32, tag="os", name=f"ot0_{x}_{o}")
                    nc.scalar.copy(ot[:], ps[:])
                    nc.sync.dma_start(OUT.ap()[P * x:P * (x + 1),
                                               W * o:W * (o + 1)], ot[:])
                blocks1 = allblk[4:8]
                pso1 = [opp.tile([P, W], F32, tag="op", name=f"op1_{b}")
                        for b in range(4)]
                oproj(pso1, blocks1, range(NPAIR))
                for ps, (x, o) in zip(pso1, blocks1):
                    ot = osb.tile([P, W], F32, tag="os", name=f"ot1_{x}_{o}")
                    nc.scalar.copy(ot[:], ps[:])
                    nc.sync.dma_start(OUT.ap()[P * x:P * (x + 1),
                                               W * o:W * (o + 1)], ot[:])

    nc.compile()
    return nc


def _get_nc():
    if "nc" not in _CACHE:
        _CACHE["nc"] = _build()
    return _CACHE["nc"]


def _esel():
    import ml_dtypes
    e = np.zeros((36, NH * 64), ml_dtypes.bfloat16)
    for h in range(NH):
        e[_zrow(h), 64 * h:64 * (h + 1)] = 1.0
    return e


def kernel(q, kv, Wq, Wkv, Wo, w=None, _trace=False):
    from concourse import bass_utils
    import ml_dtypes

    BF = ml_dtypes.bfloat16

    q = np.asarray(q, np.float32).reshape(L, DM)
    kv = np.asarray(kv, np.float32).reshape(L, DM)
    Wq = np.asarray(Wq, np.float32)
    Wkv = np.asarray(Wkv, np.float32)
    Wo = np.asarray(Wo, np.float32)

    qT = np.ascontiguousarray(q.T).astype(BF)           # [DM, L]
    kvT = np.ascontiguousarray(kv.T).astype(BF)         # [DM, L]
    WQs = np.ascontiguousarray(Wq / np.sqrt(DH)).astype(BF)   # fold 1/sqrt(d_head)
    WVK = np.ascontiguousarray(
        np.concatenate([Wkv[:, DH:], Wkv[:, :DH]], axis=1)).astype(BF)  # [Wv | Wk]
    WOb = np.ascontiguousarray(Wo).astype(BF)

    in_maps = []
    for c in range(NCORES):
        kvt_c = np.zeros((DM, YW), BF)
        lo = (c - 1) * CH
        hi = (c + 2) * CH
        src_lo, src_hi = max(lo, 0), min(hi, L)
        dst_lo = src_lo - lo
        kvt_c[:, dst_lo:dst_lo + (src_hi - src_lo)] = kvT[:, src_lo:src_hi]
        in_maps.append({
            "QT": np.ascontiguousarray(qT[:, c * CH:(c + 1) * CH]),
            "KVT": kvt_c,
            "WQ": WQs,
            "WVK": WVK,
            "WO": WOb,
            "ESEL": _esel(),
        })

    nc = _get_nc()
    res = bass_utils.run_bass_kernel_spmd(
        nc, in_maps, core_ids=list(range(NCORES)), trace=_trace)
    if _trace:
        _CACHE["last_result"] = res

    out = np.concatenate([r["OUT"] for r in res.results], axis=0)
    return out.reshape(B, L, DM).astype(np.float32)
